# revision 4
# baseline (speedup 1.0000x reference)
"""Bass/Trainium2 kernel for nn_BBBGraphConv (Bayesian GraphConv, DGL norm='both').

Computation (reference):
    W    = W_mu + W_eps * softplus(W_rho)
    bias = bias_mu + bias_eps * softplus(bias_rho)
    o    = clip(out_deg, 1)^-0.5 ; i = clip(in_deg, 1)^-0.5
    out  = segsum_dst((feat * o)[src]) @ W * i[:, None] + bias

Distribution: edges are bucketed by destination node; each of the 8 cores owns
12544 contiguous dst nodes (98 blocks x 128) and computes its output rows
exclusively. The pre-scaled node feature table (fp16) is replicated per core.
Each core gathers the source rows of its own edges with gpsimd dma_gather
(4 src windows of 25088 rows -> int16 indices; queue = window), reduces them
per destination with TensorE one-hot-mask matmuls, projects through W, applies
the dst-side norm + bias, and writes its slice of the output.

This version packs gather slots tightly: each (block, window) section is
allotted roundup16(max-over-cores count) slots (the SPMD program is shared by
all cores, so the structure is the cross-core max envelope), sections are
packed back-to-back inside each (superblock, window) gather call, and mask
groups of 128 slots may span adjacent blocks (each block runs matmuls over
exactly the groups its slots touch; foreign slots in a shared group carry
rel=-1 in that block's mask columns, so is_equal zeroes them). Call tails are
padded with idx=-1, which the dma_gather ucode trims (no descriptors
generated); the gather buffers are memset once at startup so never-written
tail slots stay finite. Masks are built transposed ([P, d, col], all operands
packed on the last axis) so the DVE runs in 2x 16-bit mode; the matmul reads
the mask with a strided free-dim AP.

Host-side work is limited to index-domain preprocessing (degree counts, sort,
bucketing, layout metadata) and the out-degree pre-scaling of the table.
"""

import numpy as np
from contextlib import ExitStack

import concourse.bass as bass
import concourse.bacc as bacc
import concourse.tile as tile
from concourse import mybir
from concourse.bass_utils import run_bass_kernel_spmd

# Problem constants (hardcoded per the harness contract)
N_NODES = 100_000
N_EDGES = 1_600_000
C = 128          # in_ch == out_ch
P = 128          # partitions
N_CORES = 8
BLK = 128        # dst nodes per block
NB = 98          # blocks per core
D_CORE = NB * BLK          # 12544 dst rows per core
N_PAD = N_CORES * D_CORE   # 100352

NW = 4           # src windows (dma_gather indices are int16)
WROWS = N_PAD // NW        # 25088 rows per window

SB_SIZES = [7] * 13 + [4, 2, 1]   # blocks per superblock (tapered tail)
SB_OFF = [sum(SB_SIZES[:i]) for i in range(len(SB_SIZES))]
N_SB = len(SB_SIZES)

TBL_DT = mybir.dt.float16
TBL_NP = np.float16

_CACHE: dict = {}


def _canonical_layout(cnt_max):
    """Build the cross-core canonical slot layout from the per-(block, window)
    max counts.  Returns a dict of compile-time structure:
      sec_alloc[b][w]   slots allotted to section (b, w) (16-aligned)
      sec_off[b][w]     slot offset of the section inside its (sb, w) call
      call_slots[s][w]  padded call size (128-aligned)
      call_real[s][w]   sum of section allotments (descriptors generated)
      base_group[s][w]  first group of call (s, w) inside the sb g_tile
      groups_sb[s]      total groups in superblock s
      blocks[b] = list of (grp_in_sb, col, sb) matmul incidences, and
      col_base[b], ncols[b]
    """
    sec_alloc = ((cnt_max + 15) // 16 * 16).astype(np.int64)   # [NB, NW]
    call_slots = np.zeros((N_SB, NW), np.int64)
    call_real = np.zeros((N_SB, NW), np.int64)
    sec_off = np.zeros((NB, NW), np.int64)
    base_group = np.zeros((N_SB, NW), np.int64)
    groups_sb = np.zeros(N_SB, np.int64)
    for s, k in enumerate(SB_SIZES):
        b0 = SB_OFF[s]
        g = 0
        for w in range(NW):
            off = 0
            for b in range(b0, b0 + k):
                sec_off[b, w] = off
                off += sec_alloc[b, w]
            call_real[s, w] = off
            call_slots[s, w] = (off + 127) // 128 * 128
            base_group[s, w] = g
            g += call_slots[s, w] // 128
        groups_sb[s] = g

    # per-block matmul incidences and mask columns
    s_of_b = np.zeros(NB, np.int64)
    for s, k in enumerate(SB_SIZES):
        s_of_b[SB_OFF[s]:SB_OFF[s] + k] = s
    blocks = []
    col_base = np.zeros(NB, np.int64)
    ncols = np.zeros(NB, np.int64)
    col = 0
    for b in range(NB):
        s = int(s_of_b[b])
        inc = []
        col_base[b] = col
        for w in range(NW):
            a = int(sec_alloc[b, w])
            if a == 0:
                continue
            lo = int(sec_off[b, w])
            glo, ghi = lo // 128, (lo + a - 1) // 128
            for g in range(glo, ghi + 1):
                inc.append((int(base_group[s, w]) + g, col))
                col += 1
        ncols[b] = col - col_base[b]
        blocks.append(inc)
    return dict(
        sec_alloc=sec_alloc, sec_off=sec_off, call_slots=call_slots,
        call_real=call_real, base_group=base_group, groups_sb=groups_sb,
        blocks=blocks, col_base=col_base, ncols=ncols, tot_cols=col,
        s_of_b=s_of_b,
    )


def _layout_key(L):
    return (
        tuple(L['sec_alloc'].ravel().tolist()),
    )


def _build_program(L):
    """Build the SPMD Bass program from the canonical layout."""
    f32 = mybir.dt.float32
    call_slots = L['call_slots']; base_group = L['base_group']
    groups_sb = L['groups_sb']; blocks = L['blocks']
    col_base = L['col_base']; ncols = L['ncols']; tot_cols = L['tot_cols']
    GMAX = int(groups_sb.max())
    NCMAX = int(ncols.max())
    slots_sb = [int(groups_sb[s]) * 128 for s in range(N_SB)]
    idx_base = np.concatenate([[0], np.cumsum([n // 16 for n in slots_sb])])

    nc = bacc.Bacc("TRN2", target_bir_lowering=False, debug=False, num_swdge_queues=4)

    table = nc.dram_tensor("table", [N_PAD, C], TBL_DT, kind="ExternalInput").ap()
    idx_t = nc.dram_tensor("idx", [P, int(idx_base[-1])], mybir.dt.int16,
                           kind="ExternalInput").ap()
    rel_t = nc.dram_tensor("rel", [P, tot_cols], TBL_DT, kind="ExternalInput").ap()
    iota_t = nc.dram_tensor("iota", [P, 128 * NCMAX], TBL_DT, kind="ExternalInput").ap()
    ivec_t = nc.dram_tensor("ivec", [P, NB], f32, kind="ExternalInput").ap()
    w_mu = nc.dram_tensor("w_mu", [C, C], f32, kind="ExternalInput").ap()
    w_rho = nc.dram_tensor("w_rho", [C, C], f32, kind="ExternalInput").ap()
    w_eps = nc.dram_tensor("w_eps", [C, C], f32, kind="ExternalInput").ap()
    b_mu = nc.dram_tensor("b_mu", [1, C], f32, kind="ExternalInput").ap()
    b_rho = nc.dram_tensor("b_rho", [1, C], f32, kind="ExternalInput").ap()
    b_eps = nc.dram_tensor("b_eps", [1, C], f32, kind="ExternalInput").ap()
    out = nc.dram_tensor("out", [D_CORE, C], f32, kind="ExternalOutput").ap()

    with tile.TileContext(nc) as tc, ExitStack() as ctx:
        const = ctx.enter_context(tc.tile_pool(name="const", bufs=1))
        gpool = ctx.enter_context(tc.tile_pool(name="gather", bufs=3))
        mpool = ctx.enter_context(tc.tile_pool(name="mask", bufs=3))
        apool = ctx.enter_context(tc.tile_pool(name="aggf", bufs=3))
        opool = ctx.enter_context(tc.tile_pool(name="ostage", bufs=3))
        pa_pool = ctx.enter_context(tc.tile_pool(name="pa", bufs=3, space="PSUM"))
        pb_pool = ctx.enter_context(tc.tile_pool(name="pb", bufs=2, space="PSUM"))
        pc_pool = ctx.enter_context(tc.tile_pool(name="pc", bufs=1, space="PSUM"))

        # --- resident inputs -------------------------------------------------
        idx_tiles = []
        for s in range(N_SB):
            t = const.tile([P, int(idx_base[s + 1] - idx_base[s])], mybir.dt.int16,
                           tag=f"idx{s}")
            nc.sync.dma_start(out=t[:], in_=idx_t[:, int(idx_base[s]):int(idx_base[s + 1])])
            idx_tiles.append(t)
        rel_sb = const.tile([P, tot_cols], TBL_DT, tag="rel")
        nc.sync.dma_start(out=rel_sb[:], in_=rel_t[:])
        iota_sb = const.tile([P, 128 * NCMAX], TBL_DT, tag="iota")
        nc.sync.dma_start(out=iota_sb[:], in_=iota_t[:])
        iota3 = iota_sb[:].rearrange("p (d j) -> p d j", j=NCMAX)
        ivec_sb = const.tile([P, NB], f32, tag="ivec")
        nc.sync.dma_start(out=ivec_sb[:], in_=ivec_t[:])

        # memset the gather pool once so call-tail slots the trimmed gather
        # never writes hold finite values (stale pool data afterwards)
        warm = []
        for i in range(3):
            t = gpool.tile([P, GMAX * C], TBL_DT, tag="g")
            nc.vector.memset(t[:], 0.0)
            warm.append(t)

        # --- W = W_mu + W_eps * softplus(W_rho) ------------------------------
        wmu_sb = const.tile([C, C], f32, tag="wmu")
        nc.sync.dma_start(out=wmu_sb[:], in_=w_mu[:])
        wrho_sb = const.tile([C, C], f32, tag="wrho")
        nc.sync.dma_start(out=wrho_sb[:], in_=w_rho[:])
        weps_sb = const.tile([C, C], f32, tag="weps")
        nc.sync.dma_start(out=weps_sb[:], in_=w_eps[:])
        w_sp = const.tile([C, C], f32, tag="wsp")
        nc.scalar.activation(w_sp[:], wrho_sb[:], mybir.ActivationFunctionType.Exp)
        nc.scalar.activation(w_sp[:], w_sp[:], mybir.ActivationFunctionType.Ln, bias=1.0)
        w_sb = const.tile([C, C], f32, tag="w")
        nc.vector.tensor_tensor(out=w_sb[:], in0=weps_sb[:], in1=w_sp[:], op=mybir.AluOpType.mult)
        nc.vector.tensor_tensor(out=w_sb[:], in0=w_sb[:], in1=wmu_sb[:], op=mybir.AluOpType.add)

        # --- bias tile [P, C]: every partition row holds the bias vector -----
        bmu_sb = const.tile([1, C], f32, tag="bmu")
        nc.sync.dma_start(out=bmu_sb[:], in_=b_mu[:])
        brho_sb = const.tile([1, C], f32, tag="brho")
        nc.sync.dma_start(out=brho_sb[:], in_=b_rho[:])
        beps_sb = const.tile([1, C], f32, tag="beps")
        nc.sync.dma_start(out=beps_sb[:], in_=b_eps[:])
        b_sp = const.tile([1, C], f32, tag="bsp")
        nc.scalar.activation(b_sp[:], brho_sb[:], mybir.ActivationFunctionType.Exp)
        nc.scalar.activation(b_sp[:], b_sp[:], mybir.ActivationFunctionType.Ln, bias=1.0)
        b_vec = const.tile([1, C], f32, tag="bvec")
        nc.vector.tensor_tensor(out=b_vec[:], in0=beps_sb[:], in1=b_sp[:], op=mybir.AluOpType.mult)
        nc.vector.tensor_tensor(out=b_vec[:], in0=b_vec[:], in1=bmu_sb[:], op=mybir.AluOpType.add)
        ones_1p = const.tile([1, C], f32, tag="ones")
        nc.vector.memset(ones_1p[:], 1.0)
        p_bias = pc_pool.tile([P, C], f32, tag="pbias")
        nc.tensor.matmul(out=p_bias[:], lhsT=ones_1p[:], rhs=b_vec[:], start=True, stop=True)
        bias_tile = const.tile([P, C], f32, tag="bias")
        nc.vector.tensor_copy(out=bias_tile[:], in_=p_bias[:])

        # --- main loop over superblocks --------------------------------------
        first_g = {0: warm[0], 1: warm[1], 2: warm[2]}
        for s in range(N_SB):
            k_sb = SB_SIZES[s]
            if s < 3:
                g_tile = first_g[s]          # memset tiles double as first bufs
            else:
                g_tile = gpool.tile([P, GMAX * C], TBL_DT, tag="g")
            g3 = g_tile[:].rearrange("p (g c) -> p g c", c=C)
            for w in range(NW):
                cs = int(call_slots[s, w])
                if cs == 0:
                    continue
                bg = int(base_group[s, w])
                i0 = int(np.sum([call_slots[s, w2] // 16 for w2 in range(w)]))
                nc.gpsimd.dma_gather(
                    out_ap=g3[:, bg:bg + cs // 128, :],
                    in_ap=table[w * WROWS:(w + 1) * WROWS, :],
                    idxs_ap=idx_tiles[s][:, i0:i0 + cs // 16],
                    num_idxs=cs,
                    num_idxs_reg=cs,
                    elem_size=C,
                    queue_num=w,
                    single_packet=False,
                )
            ostage = opool.tile([P, k_sb * C], f32, tag="ostage")
            for bb in range(k_sb):
                b = SB_OFF[s] + bb
                nb_cols = int(ncols[b])
                if nb_cols == 0:
                    # empty block: bias only
                    nc.vector.tensor_copy(out=ostage[:, bb * C:(bb + 1) * C],
                                          in_=bias_tile[:])
                    continue
                cb = int(col_base[b])
                mask = mpool.tile([P, 128 * nb_cols], TBL_DT, tag="mask")
                mask3 = mask[:].rearrange("p (d j) -> p d j", j=nb_cols)
                rel_b = rel_sb[:, cb:cb + nb_cols].unsqueeze(1).to_broadcast(
                    [P, 128, nb_cols]
                )
                nc.vector.tensor_tensor(
                    out=mask3,
                    in0=iota3[:, :, 0:nb_cols],
                    in1=rel_b,
                    op=mybir.AluOpType.is_equal,
                )
                pa = pa_pool.tile([C, BLK], f32, tag="pa")
                inc = blocks[b]
                for j, (grp, col) in enumerate(inc):
                    nc.tensor.matmul(
                        out=pa[:],
                        lhsT=g_tile[:, grp * C:(grp + 1) * C],
                        rhs=mask3[:, :, col - cb],
                        start=(j == 0),
                        stop=(j == len(inc) - 1),
                    )
                agg = apool.tile([C, BLK], f32, tag="agg")
                nc.scalar.activation(agg[:], pa[:], mybir.ActivationFunctionType.Copy)
                pb = pb_pool.tile([BLK, C], f32, tag="pb")
                nc.tensor.matmul(out=pb[:], lhsT=agg[:], rhs=w_sb[:], start=True, stop=True)
                nc.vector.scalar_tensor_tensor(
                    out=ostage[:, bb * C:(bb + 1) * C],
                    in0=pb[:],
                    scalar=ivec_sb[:, b:b + 1],
                    in1=bias_tile[:],
                    op0=mybir.AluOpType.mult,
                    op1=mybir.AluOpType.add,
                )
            dram_view = out[SB_OFF[s] * BLK:(SB_OFF[s] + k_sb) * BLK, :].rearrange(
                "(bb p) c -> p bb c", p=P
            )
            nc.sync.dma_start(
                out=dram_view, in_=ostage[:].rearrange("p (bb c) -> p bb c", bb=k_sb)
            )

    nc.compile()
    return nc


def _preprocess(feat, src, dst, W_mu, W_rho, bias_mu, bias_rho, W_eps, bias_eps):
    """Index-domain preprocessing + table pre-scaling. Returns (in_maps, layout)."""
    src = np.asarray(src).astype(np.int64)
    dst = np.asarray(dst).astype(np.int64)
    feat = np.asarray(feat, dtype=np.float32)

    out_deg = np.bincount(src, minlength=N_NODES).astype(np.float32)
    o = 1.0 / np.sqrt(np.maximum(out_deg, 1.0))
    in_deg = np.bincount(dst, minlength=N_NODES)
    ivec_full = (1.0 / np.sqrt(np.maximum(in_deg, 1.0))).astype(np.float32)

    table = np.zeros((N_PAD, C), TBL_NP)
    table[:N_NODES] = (feat * o[:, None]).astype(TBL_NP)

    blk = dst >> 7
    win = src // WROWS
    core = blk // NB
    b_loc = blk % NB

    cnt = np.zeros((N_CORES, NB, NW), np.int64)
    np.add.at(cnt, (core, b_loc, win), 1)
    L = _canonical_layout(cnt.max(axis=0))

    sec_alloc = L['sec_alloc']; sec_off = L['sec_off']
    call_slots = L['call_slots']; call_real = L['call_real']
    base_group = L['base_group']; groups_sb = L['groups_sb']
    col_base = L['col_base']; s_of_b = L['s_of_b']
    slots_sb = groups_sb * 128
    sb_slot_base = np.concatenate([[0], np.cumsum(slots_sb)])
    tot_slots = int(sb_slot_base[-1])

    # global slot position of each edge:
    #   slot = sb_slot_base[s] + call_base_in_sb[s, w] + sec_off[b, w] + rank
    call_base_in_sb = np.zeros((N_SB, NW), np.int64)
    for s in range(N_SB):
        acc = 0
        for w in range(NW):
            call_base_in_sb[s, w] = acc
            acc += int(call_slots[s, w])

    order = np.lexsort((src, win + NW * blk))
    sblk = blk[order]; swin = win[order]; ss = src[order]; sd = dst[order]
    score = sblk // NB; sb_loc = sblk % NB
    sec_id = (score * NB + sb_loc) * NW + swin
    sec_cnt_flat = np.bincount(sec_id, minlength=N_CORES * NB * NW)
    starts = np.zeros(len(sec_cnt_flat) + 1, np.int64)
    np.cumsum(sec_cnt_flat, out=starts[1:])
    rank = np.arange(len(ss), dtype=np.int64) - starts[sec_id]

    s_arr = s_of_b[sb_loc]
    slot_in_sb = (call_base_in_sb[s_arr, swin] + sec_off[sb_loc, swin] + rank)
    slot = sb_slot_base[s_arr] + slot_in_sb

    # idx values: 0 = section padding (gathers window row 0, masked out),
    # -1 = call-tail padding (descriptor generation skipped by the ucode)
    idx_all = np.zeros((N_CORES, tot_slots), np.int16)
    for s in range(N_SB):
        for w in range(NW):
            lo = int(sb_slot_base[s] + call_base_in_sb[s, w] + call_real[s, w])
            hi = int(sb_slot_base[s] + call_base_in_sb[s, w] + call_slots[s, w])
            idx_all[:, lo:hi] = -1
    idx_all[score, slot] = (ss - swin * WROWS).astype(np.int16)

    # rel: [core, P, tot_cols]; default -1, edge lanes carry dst & 127.
    # col of an edge: block's col_base + index of (w, group) in its incidence
    # list.  Since incidences are ordered (w asc, g asc) and contiguous per
    # (b, w), col = col_base[b] + cols_before_w[b, w] + (group - glo).
    cols_before = np.zeros((NB, NW), np.int64)
    glo_arr = np.zeros((NB, NW), np.int64)
    for b in range(NB):
        acc = 0
        for w in range(NW):
            a = int(sec_alloc[b, w])
            cols_before[b, w] = acc
            if a == 0:
                continue
            lo = int(sec_off[b, w])
            glo, ghi = lo // 128, (lo + a - 1) // 128
            glo_arr[b, w] = glo
            acc += ghi - glo + 1

    pos_in_call = sec_off[sb_loc, swin] + rank
    g_in_call = pos_in_call // 128
    lane = pos_in_call % 128
    ecol = (col_base[sb_loc] + cols_before[sb_loc, swin]
            + (g_in_call - glo_arr[sb_loc, swin]))
    rel_all = np.full((N_CORES, P, L['tot_cols']), -1.0, TBL_NP)
    rel_all[score, lane, ecol] = (sd & 127).astype(TBL_NP)

    ivec_pad = np.ones(N_PAD, np.float32)
    ivec_pad[:N_NODES] = ivec_full

    NCMAX = int(L['ncols'].max())
    iota_np = np.repeat(np.arange(128, dtype=TBL_NP), NCMAX)[None, :].repeat(P, 0)
    iota_np = np.ascontiguousarray(iota_np)

    cc = np.ascontiguousarray
    in_maps = []
    for c in range(N_CORES):
        idx_c = idx_all[c]
        idx_tile = np.tile(idx_c.reshape(-1, 16).T, (8, 1))
        in_maps.append({
            "table": table,
            "iota": iota_np,
            "idx": cc(idx_tile),
            "rel": cc(rel_all[c]),
            "ivec": cc(ivec_pad[c * D_CORE:(c + 1) * D_CORE].reshape(NB, P).T),
            "w_mu": np.asarray(W_mu, np.float32),
            "w_rho": np.asarray(W_rho, np.float32),
            "w_eps": np.asarray(W_eps, np.float32),
            "b_mu": np.asarray(bias_mu, np.float32).reshape(1, C),
            "b_rho": np.asarray(bias_rho, np.float32).reshape(1, C),
            "b_eps": np.asarray(bias_eps, np.float32).reshape(1, C),
        })
    return in_maps, L


def kernel(**inputs) -> np.ndarray:
    in_maps, L = _preprocess(**inputs)
    key = _layout_key(L)
    if key not in _CACHE:
        _CACHE[key] = _build_program(L)
    nc = _CACHE[key]
    res = run_bass_kernel_spmd(nc, in_maps, core_ids=list(range(N_CORES)))
    parts = [res.results[c]["out"] for c in range(N_CORES)]
    return np.concatenate(parts, axis=0)[:N_NODES]


# revision 11
# speedup vs baseline: 1.4166x; 1.4166x over previous
"""Bass/Trainium2 kernel for nn_BBBGraphConv (Bayesian GraphConv, DGL norm='both').

Computation (reference):
    W    = W_mu + W_eps * softplus(W_rho)
    bias = bias_mu + bias_eps * softplus(bias_rho)
    o    = clip(out_deg, 1)^-0.5 ; i = clip(in_deg, 1)^-0.5
    out  = segsum_dst((feat * o)[src]) @ W * i[:, None] + bias

Distribution: edges are bucketed by destination node; each of the 8 cores owns
12544 contiguous dst nodes (98 blocks x 128) and computes its output rows
exclusively. The pre-scaled node feature table (fp16) is replicated per core.
Each core gathers the source rows of its own edges with gpsimd dma_gather
(4 src windows of 25088 rows -> int16 indices; queue = window), reduces them
per destination with TensorE one-hot-mask matmuls, projects through W, applies
the dst-side norm + bias, and writes its slice of the output.

This version packs gather slots tightly: each (block, window) section is
allotted roundup16(max-over-cores count) slots (the SPMD program is shared by
all cores, so the structure is the cross-core max envelope), sections are
packed back-to-back inside each (superblock, window) gather call, and mask
groups of 128 slots may span adjacent blocks (each block runs matmuls over
exactly the groups its slots touch; foreign slots in a shared group carry
rel=-1 in that block's mask columns, so is_equal zeroes them). Call tails are
padded with idx=-1, which the dma_gather ucode trims (no descriptors
generated); the gather buffers are memset once at startup so never-written
tail slots stay finite. Masks are built transposed ([P, d, col], all operands
packed on the last axis) so the DVE runs in 2x 16-bit mode; the matmul reads
the mask with a strided free-dim AP.

Host-side work is limited to index-domain preprocessing (degree counts, sort,
bucketing, layout metadata) and the out-degree pre-scaling of the table.
"""

import numpy as np
from contextlib import ExitStack

import concourse.bass as bass
import concourse.bacc as bacc
import concourse.tile as tile
from concourse import mybir
from concourse.bass_utils import run_bass_kernel_spmd

# Problem constants (hardcoded per the harness contract)
N_NODES = 100_000
N_EDGES = 1_600_000
C = 128          # in_ch == out_ch
P = 128          # partitions
N_CORES = 8
BLK = 128        # dst nodes per block
NB = 98          # blocks per core
D_CORE = NB * BLK          # 12544 dst rows per core
N_PAD = N_CORES * D_CORE   # 100352

NW = 4           # src windows (dma_gather indices are int16)
WROWS = N_PAD // NW        # 25088 rows per window

SB_SIZES = [7] * 13 + [4, 2, 1]   # blocks per superblock (tapered tail)
SB_OFF = [sum(SB_SIZES[:i]) for i in range(len(SB_SIZES))]
N_SB = len(SB_SIZES)

TBL_DT = mybir.dt.float16
TBL_NP = np.float16

_CACHE: dict = {}


def _canonical_layout(cnt_max):
    """Build the cross-core canonical slot layout from the per-(block, window)
    max counts.  Returns a dict of compile-time structure:
      sec_alloc[b][w]   slots allotted to section (b, w) (16-aligned)
      sec_off[b][w]     slot offset of the section inside its (sb, w) call
      call_slots[s][w]  padded call size (128-aligned)
      call_real[s][w]   sum of section allotments (descriptors generated)
      base_group[s][w]  first group of call (s, w) inside the sb g_tile
      groups_sb[s]      total groups in superblock s
      blocks[b] = list of (grp_in_sb, col, sb) matmul incidences, and
      col_base[b], ncols[b]
    """
    sec_alloc = ((cnt_max + 15) // 16 * 16).astype(np.int64)   # [NB, NW]
    call_slots = np.zeros((N_SB, NW), np.int64)
    call_real = np.zeros((N_SB, NW), np.int64)
    sec_off = np.zeros((NB, NW), np.int64)
    base_group = np.zeros((N_SB, NW), np.int64)
    groups_sb = np.zeros(N_SB, np.int64)
    for s, k in enumerate(SB_SIZES):
        b0 = SB_OFF[s]
        g = 0
        for w in range(NW):
            off = 0
            for b in range(b0, b0 + k):
                sec_off[b, w] = off
                off += sec_alloc[b, w]
            call_real[s, w] = off
            call_slots[s, w] = (off + 127) // 128 * 128
            base_group[s, w] = g
            g += call_slots[s, w] // 128
        groups_sb[s] = g

    # per-block matmul incidences and mask columns
    s_of_b = np.zeros(NB, np.int64)
    for s, k in enumerate(SB_SIZES):
        s_of_b[SB_OFF[s]:SB_OFF[s] + k] = s
    blocks = []
    col_base = np.zeros(NB, np.int64)
    ncols = np.zeros(NB, np.int64)
    col = 0
    for b in range(NB):
        s = int(s_of_b[b])
        inc = []
        col_base[b] = col
        for w in range(NW):
            a = int(sec_alloc[b, w])
            if a == 0:
                continue
            lo = int(sec_off[b, w])
            glo, ghi = lo // 128, (lo + a - 1) // 128
            for g in range(glo, ghi + 1):
                inc.append((int(base_group[s, w]) + g, col))
                col += 1
        ncols[b] = col - col_base[b]
        blocks.append(inc)
    return dict(
        sec_alloc=sec_alloc, sec_off=sec_off, call_slots=call_slots,
        call_real=call_real, base_group=base_group, groups_sb=groups_sb,
        blocks=blocks, col_base=col_base, ncols=ncols, tot_cols=col,
        s_of_b=s_of_b,
    )


def _layout_key(L):
    return (
        tuple(L['sec_alloc'].ravel().tolist()),
    )


def _build_program(L):
    """Build the SPMD Bass program from the canonical layout."""
    f32 = mybir.dt.float32
    call_slots = L['call_slots']; base_group = L['base_group']
    groups_sb = L['groups_sb']; blocks = L['blocks']
    col_base = L['col_base']; ncols = L['ncols']; tot_cols = L['tot_cols']
    GMAX = int(groups_sb.max())
    slots_sb = [int(groups_sb[s]) * 128 for s in range(N_SB)]
    idx_base = np.concatenate([[0], np.cumsum([n // 16 for n in slots_sb])])

    nc = bacc.Bacc("TRN2", target_bir_lowering=False, debug=False, num_swdge_queues=4)

    table = nc.dram_tensor("table", [N_PAD, C], TBL_DT, kind="ExternalInput").ap()
    idx_t = nc.dram_tensor("idx", [P, int(idx_base[-1])], mybir.dt.int16,
                           kind="ExternalInput").ap()
    rel_t = nc.dram_tensor("rel", [P, tot_cols], TBL_DT, kind="ExternalInput").ap()
    iota_t = nc.dram_tensor("iota", [P, 128], TBL_DT, kind="ExternalInput").ap()
    ivec_t = nc.dram_tensor("ivec", [P, NB], f32, kind="ExternalInput").ap()
    w_mu = nc.dram_tensor("w_mu", [C, C], f32, kind="ExternalInput").ap()
    w_rho = nc.dram_tensor("w_rho", [C, C], f32, kind="ExternalInput").ap()
    w_eps = nc.dram_tensor("w_eps", [C, C], f32, kind="ExternalInput").ap()
    b_mu = nc.dram_tensor("b_mu", [1, C], f32, kind="ExternalInput").ap()
    b_rho = nc.dram_tensor("b_rho", [1, C], f32, kind="ExternalInput").ap()
    b_eps = nc.dram_tensor("b_eps", [1, C], f32, kind="ExternalInput").ap()
    out = nc.dram_tensor("out", [D_CORE, C], f32, kind="ExternalOutput").ap()

    with tile.TileContext(nc) as tc, ExitStack() as ctx:
        const = ctx.enter_context(tc.tile_pool(name="const", bufs=1))
        gpool = ctx.enter_context(tc.tile_pool(name="gather", bufs=3))
        mpool = ctx.enter_context(tc.tile_pool(name="mask", bufs=3))
        apool = ctx.enter_context(tc.tile_pool(name="aggf", bufs=3))
        opool = ctx.enter_context(tc.tile_pool(name="ostage", bufs=3))
        pa_pool = ctx.enter_context(tc.tile_pool(name="pa", bufs=3, space="PSUM"))
        pb_pool = ctx.enter_context(tc.tile_pool(name="pb", bufs=2, space="PSUM"))
        pc_pool = ctx.enter_context(tc.tile_pool(name="pc", bufs=1, space="PSUM"))

        # --- resident inputs -------------------------------------------------
        idx_tiles = []
        for s in range(N_SB):
            t = const.tile([P, int(idx_base[s + 1] - idx_base[s])], mybir.dt.int16,
                           tag=f"idx{s}")
            nc.sync.dma_start(out=t[:], in_=idx_t[:, int(idx_base[s]):int(idx_base[s + 1])])
            idx_tiles.append(t)
        rel_sb = const.tile([P, tot_cols], TBL_DT, tag="rel")
        nc.sync.dma_start(out=rel_sb[:], in_=rel_t[:])
        iota_sb = const.tile([P, 128], TBL_DT, tag="iota")
        nc.sync.dma_start(out=iota_sb[:], in_=iota_t[:])
        ivec_sb = const.tile([P, NB], f32, tag="ivec")
        nc.sync.dma_start(out=ivec_sb[:], in_=ivec_t[:])

        # --- W = W_mu + W_eps * softplus(W_rho) ------------------------------
        wmu_sb = const.tile([C, C], f32, tag="wmu")
        nc.sync.dma_start(out=wmu_sb[:], in_=w_mu[:])
        wrho_sb = const.tile([C, C], f32, tag="wrho")
        nc.sync.dma_start(out=wrho_sb[:], in_=w_rho[:])
        weps_sb = const.tile([C, C], f32, tag="weps")
        nc.sync.dma_start(out=weps_sb[:], in_=w_eps[:])
        w_sp = const.tile([C, C], f32, tag="wsp")
        nc.scalar.activation(w_sp[:], wrho_sb[:], mybir.ActivationFunctionType.Exp)
        nc.scalar.activation(w_sp[:], w_sp[:], mybir.ActivationFunctionType.Ln, bias=1.0)
        w_sb = const.tile([C, C], f32, tag="w")
        nc.vector.tensor_tensor(out=w_sb[:], in0=weps_sb[:], in1=w_sp[:], op=mybir.AluOpType.mult)
        nc.vector.tensor_tensor(out=w_sb[:], in0=w_sb[:], in1=wmu_sb[:], op=mybir.AluOpType.add)

        # --- bias tile [P, C]: every partition row holds the bias vector -----
        bmu_sb = const.tile([1, C], f32, tag="bmu")
        nc.sync.dma_start(out=bmu_sb[:], in_=b_mu[:])
        brho_sb = const.tile([1, C], f32, tag="brho")
        nc.sync.dma_start(out=brho_sb[:], in_=b_rho[:])
        beps_sb = const.tile([1, C], f32, tag="beps")
        nc.sync.dma_start(out=beps_sb[:], in_=b_eps[:])
        b_sp = const.tile([1, C], f32, tag="bsp")
        nc.scalar.activation(b_sp[:], brho_sb[:], mybir.ActivationFunctionType.Exp)
        nc.scalar.activation(b_sp[:], b_sp[:], mybir.ActivationFunctionType.Ln, bias=1.0)
        b_vec = const.tile([1, C], f32, tag="bvec")
        nc.vector.tensor_tensor(out=b_vec[:], in0=beps_sb[:], in1=b_sp[:], op=mybir.AluOpType.mult)
        nc.vector.tensor_tensor(out=b_vec[:], in0=b_vec[:], in1=bmu_sb[:], op=mybir.AluOpType.add)
        ones_1p = const.tile([1, C], f32, tag="ones")
        nc.vector.memset(ones_1p[:], 1.0)
        p_bias = pc_pool.tile([P, C], f32, tag="pbias")
        nc.tensor.matmul(out=p_bias[:], lhsT=ones_1p[:], rhs=b_vec[:], start=True, stop=True)
        bias_tile = const.tile([P, C], f32, tag="bias")
        nc.vector.tensor_copy(out=bias_tile[:], in_=p_bias[:])

        # --- main loop over superblocks --------------------------------------
        for s in range(N_SB):
            k_sb = SB_SIZES[s]
            g_tile = gpool.tile([P, GMAX * C], TBL_DT, tag="g")
            g3 = g_tile[:].rearrange("p (g c) -> p g c", c=C)
            for w in range(NW):
                cs = int(call_slots[s, w])
                if cs == 0:
                    continue
                bg = int(base_group[s, w])
                i0 = int(np.sum([call_slots[s, w2] // 16 for w2 in range(w)]))
                nc.gpsimd.dma_gather(
                    out_ap=g3[:, bg:bg + cs // 128, :],
                    in_ap=table[w * WROWS:(w + 1) * WROWS, :],
                    idxs_ap=idx_tiles[s][:, i0:i0 + cs // 16],
                    num_idxs=cs,
                    num_idxs_reg=cs,
                    elem_size=C,
                    queue_num=w,
                    single_packet=False,
                )
            ostage = opool.tile([P, k_sb * C], f32, tag="ostage")
            for bb in range(k_sb):
                b = SB_OFF[s] + bb
                nb_cols = int(ncols[b])
                if nb_cols == 0:
                    # empty block: bias only
                    nc.vector.tensor_copy(out=ostage[:, bb * C:(bb + 1) * C],
                                          in_=bias_tile[:])
                    continue
                cb = int(col_base[b])
                mask = mpool.tile([P, nb_cols * 128], TBL_DT, tag="mask")
                mask3 = mask[:].rearrange("p (j d) -> p j d", j=nb_cols)
                rel_b = rel_sb[:, cb:cb + nb_cols].unsqueeze(2).to_broadcast(
                    [P, nb_cols, 128]
                )
                iota_b = iota_sb[:].unsqueeze(1).to_broadcast([P, nb_cols, 128])
                nc.vector.tensor_tensor(
                    out=mask3,
                    in0=iota_b,
                    in1=rel_b,
                    op=mybir.AluOpType.is_equal,
                )
                pa = pa_pool.tile([C, BLK], f32, tag="pa")
                inc = blocks[b]
                for j, (grp, col) in enumerate(inc):
                    jj = col - cb
                    nc.tensor.matmul(
                        out=pa[:],
                        lhsT=g_tile[:, grp * C:(grp + 1) * C],
                        rhs=mask[:, jj * 128:(jj + 1) * 128],
                        start=(j == 0),
                        stop=(j == len(inc) - 1),
                    )
                agg = apool.tile([C, BLK], f32, tag="agg")
                nc.scalar.activation(agg[:], pa[:], mybir.ActivationFunctionType.Copy)
                pb = pb_pool.tile([BLK, C], f32, tag="pb")
                nc.tensor.matmul(out=pb[:], lhsT=agg[:], rhs=w_sb[:], start=True, stop=True)
                nc.vector.scalar_tensor_tensor(
                    out=ostage[:, bb * C:(bb + 1) * C],
                    in0=pb[:],
                    scalar=ivec_sb[:, b:b + 1],
                    in1=bias_tile[:],
                    op0=mybir.AluOpType.mult,
                    op1=mybir.AluOpType.add,
                )
            dram_view = out[SB_OFF[s] * BLK:(SB_OFF[s] + k_sb) * BLK, :].rearrange(
                "(bb p) c -> p bb c", p=P
            )
            nc.sync.dma_start(
                out=dram_view, in_=ostage[:].rearrange("p (bb c) -> p bb c", bb=k_sb)
            )

    nc.compile()
    return nc


def _preprocess(feat, src, dst, W_mu, W_rho, bias_mu, bias_rho, W_eps, bias_eps):
    """Index-domain preprocessing + table pre-scaling. Returns (in_maps, layout)."""
    src = np.asarray(src).astype(np.int64)
    dst = np.asarray(dst).astype(np.int64)
    feat = np.asarray(feat, dtype=np.float32)

    out_deg = np.bincount(src, minlength=N_NODES).astype(np.float32)
    o = 1.0 / np.sqrt(np.maximum(out_deg, 1.0))
    in_deg = np.bincount(dst, minlength=N_NODES)
    ivec_full = (1.0 / np.sqrt(np.maximum(in_deg, 1.0))).astype(np.float32)

    table = np.zeros((N_PAD, C), TBL_NP)
    table[:N_NODES] = (feat * o[:, None]).astype(TBL_NP)

    blk = dst >> 7
    win = src // WROWS
    core = blk // NB
    b_loc = blk % NB

    cnt = np.zeros((N_CORES, NB, NW), np.int64)
    np.add.at(cnt, (core, b_loc, win), 1)
    L = _canonical_layout(cnt.max(axis=0))

    sec_alloc = L['sec_alloc']; sec_off = L['sec_off']
    call_slots = L['call_slots']; call_real = L['call_real']
    base_group = L['base_group']; groups_sb = L['groups_sb']
    col_base = L['col_base']; s_of_b = L['s_of_b']
    slots_sb = groups_sb * 128
    sb_slot_base = np.concatenate([[0], np.cumsum(slots_sb)])
    tot_slots = int(sb_slot_base[-1])

    # global slot position of each edge:
    #   slot = sb_slot_base[s] + call_base_in_sb[s, w] + sec_off[b, w] + rank
    call_base_in_sb = np.zeros((N_SB, NW), np.int64)
    for s in range(N_SB):
        acc = 0
        for w in range(NW):
            call_base_in_sb[s, w] = acc
            acc += int(call_slots[s, w])

    order = np.lexsort((src, win + NW * blk))
    sblk = blk[order]; swin = win[order]; ss = src[order]; sd = dst[order]
    score = sblk // NB; sb_loc = sblk % NB
    sec_id = (score * NB + sb_loc) * NW + swin
    sec_cnt_flat = np.bincount(sec_id, minlength=N_CORES * NB * NW)
    starts = np.zeros(len(sec_cnt_flat) + 1, np.int64)
    np.cumsum(sec_cnt_flat, out=starts[1:])
    rank = np.arange(len(ss), dtype=np.int64) - starts[sec_id]

    s_arr = s_of_b[sb_loc]
    slot_in_sb = (call_base_in_sb[s_arr, swin] + sec_off[sb_loc, swin] + rank)
    slot = sb_slot_base[s_arr] + slot_in_sb

    # idx values: 0 = section padding (gathers window row 0, masked out),
    # -1 = call-tail padding (descriptor generation skipped by the ucode).
    # The first 3 superblocks keep 0-pads everywhere so the three rotating
    # gather buffers are fully written on first use (no stale/NaN SBUF).
    idx_all = np.zeros((N_CORES, tot_slots), np.int16)
    for s in range(3, N_SB):
        for w in range(NW):
            lo = int(sb_slot_base[s] + call_base_in_sb[s, w] + call_real[s, w])
            hi = int(sb_slot_base[s] + call_base_in_sb[s, w] + call_slots[s, w])
            idx_all[:, lo:hi] = -1
    idx_all[score, slot] = (ss - swin * WROWS).astype(np.int16)

    # rel: [core, P, tot_cols]; default -1, edge lanes carry dst & 127.
    # col of an edge: block's col_base + index of (w, group) in its incidence
    # list.  Since incidences are ordered (w asc, g asc) and contiguous per
    # (b, w), col = col_base[b] + cols_before_w[b, w] + (group - glo).
    cols_before = np.zeros((NB, NW), np.int64)
    glo_arr = np.zeros((NB, NW), np.int64)
    for b in range(NB):
        acc = 0
        for w in range(NW):
            a = int(sec_alloc[b, w])
            cols_before[b, w] = acc
            if a == 0:
                continue
            lo = int(sec_off[b, w])
            glo, ghi = lo // 128, (lo + a - 1) // 128
            glo_arr[b, w] = glo
            acc += ghi - glo + 1

    pos_in_call = sec_off[sb_loc, swin] + rank
    g_in_call = pos_in_call // 128
    lane = pos_in_call % 128
    ecol = (col_base[sb_loc] + cols_before[sb_loc, swin]
            + (g_in_call - glo_arr[sb_loc, swin]))
    rel_all = np.full((N_CORES, P, L['tot_cols']), -1.0, TBL_NP)
    rel_all[score, lane, ecol] = (sd & 127).astype(TBL_NP)

    ivec_pad = np.ones(N_PAD, np.float32)
    ivec_pad[:N_NODES] = ivec_full

    iota_np = np.ascontiguousarray(
        np.arange(128, dtype=TBL_NP)[None, :].repeat(P, 0))

    cc = np.ascontiguousarray
    in_maps = []
    for c in range(N_CORES):
        idx_c = idx_all[c]
        idx_tile = np.tile(idx_c.reshape(-1, 16).T, (8, 1))
        in_maps.append({
            "table": table,
            "iota": iota_np,
            "idx": cc(idx_tile),
            "rel": cc(rel_all[c]),
            "ivec": cc(ivec_pad[c * D_CORE:(c + 1) * D_CORE].reshape(NB, P).T),
            "w_mu": np.asarray(W_mu, np.float32),
            "w_rho": np.asarray(W_rho, np.float32),
            "w_eps": np.asarray(W_eps, np.float32),
            "b_mu": np.asarray(bias_mu, np.float32).reshape(1, C),
            "b_rho": np.asarray(bias_rho, np.float32).reshape(1, C),
            "b_eps": np.asarray(bias_eps, np.float32).reshape(1, C),
        })
    return in_maps, L


def kernel(**inputs) -> np.ndarray:
    in_maps, L = _preprocess(**inputs)
    key = _layout_key(L)
    if key not in _CACHE:
        _CACHE[key] = _build_program(L)
    nc = _CACHE[key]
    res = run_bass_kernel_spmd(nc, in_maps, core_ids=list(range(N_CORES)))
    parts = [res.results[c]["out"] for c in range(N_CORES)]
    return np.concatenate(parts, axis=0)[:N_NODES]
